# revision 7
# baseline (speedup 1.0000x reference)
"""Trainium2 Bass kernel for nn_BigramBaseline: causal mean pooling over
embedding-gathered rows.

  logits[b*T + t, :] = mean_{s<=t} emb[idx[b, s], :]

Strategy (data-parallel over batch, one batch row per core):
  - emb is cast to bf16 on host; the device gathers bf16 rows and writes
    bf16 outputs (upcast to f32 on host). Rounding error ~0.3% rel, well
    under the 2e-2 gate, and it halves HBM traffic both ways: 64 MiB ->
    32 MiB per core, which is what matters in this memory-bound regime.
  - per 128-token block: indirect-DMA gather of 128 emb rows -> SBUF
    tile [128, V] (partition = token within block)
  - in-block causal prefix sum via PE matmul with a lower-triangular
    ones matrix (lhsT = upper-triangular incl. diag)
  - cross-block carry kept resident in PSUM: after emitting the block's
    prefix sums, a second matmul with the strict complement mask adds
    the rest of the block's column-sums, turning the PSUM bank into
    carry_{k+1} broadcast over all 128 partitions
  - scale by 1/(t+1) during the PSUM->SBUF copy (per-partition scale
    operand); chunks 0-3 go through the scalar engine, 4-7 through the
    vector engine, splitting the copy load
  - tril/strict matmuls are batched per block (all 8 tril, then all 8
    strict) so the PE swaps weights twice per block instead of 16 times
  - one full-width DMA per block in each direction (128 x 8KB
    descriptors): bigger descriptors amortize per-descriptor overhead
    and halve the DGE generation ops per block
"""

import numpy as np
import ml_dtypes

B, T, V = 8, 2048, 4096
P = 128
CHUNK = 512
N_CORES = 8


def build_bass(t=T, v=V):
    import concourse.bacc as bacc
    import concourse.bass as bass
    import concourse.tile as tile
    from concourse import mybir

    nblk = t // P
    chunk = min(CHUNK, v)
    nchunk = v // chunk

    mm_dt = mybir.dt.bfloat16

    # Bacc (not plain Bass): its finalize() runs generate_event_semaphores,
    # which splits multi-sem waits — walrus codegen only fits one sync wait
    # per instruction.
    nc = bacc.Bacc(trn_type="TRN2")
    emb = nc.declare_dram_parameter("emb", [v, v], mm_dt, isOutput=False)
    idx = nc.declare_dram_parameter("idx", [P, nblk], mybir.dt.int32, isOutput=False)
    invd = nc.declare_dram_parameter("invd", [P, nblk], mybir.dt.float32, isOutput=False)
    # masks[:, 0:P]  = lhsT for the in-block prefix sum: m[s, p] = 1 iff s <= p
    # masks[:, P:2P] = lhsT for the carry update:        m[s, p] = 1 iff s > p
    masks = nc.declare_dram_parameter("masks", [P, 2 * P], mm_dt, isOutput=False)
    out = nc.declare_dram_parameter("out", [t, v], mm_dt, isOutput=True)

    with tile.TileContext(nc) as tc:
        with (
            tc.tile_pool(name="const", bufs=1) as cpool,
            tc.tile_pool(name="x", bufs=6) as xpool,
            tc.tile_pool(name="o", bufs=6) as opool,
            tc.tile_pool(name="acc", bufs=1, space="PSUM") as ppool,
        ):
            idx_sb = cpool.tile([P, nblk], mybir.dt.int32)
            nc.sync.dma_start(out=idx_sb[:], in_=idx[:])
            invd_sb = cpool.tile([P, nblk], mybir.dt.float32)
            nc.sync.dma_start(out=invd_sb[:], in_=invd[:])
            masks_sb = cpool.tile([P, 2 * P], mm_dt)
            nc.sync.dma_start(out=masks_sb[:], in_=masks[:])
            trilT_sb = masks_sb[:, 0:P]
            strictT_sb = masks_sb[:, P : 2 * P]

            acc = [
                ppool.tile([P, chunk], mybir.dt.float32, name=f"acc{c}", tag=f"acc{c}")
                for c in range(nchunk)
            ]

            # Walrus only fits ONE sync wait per engine instruction, so each
            # engine pre-absorbs its constant-DMA wait in a tiny warm-up op;
            # the real ops then carry only their single data-flow wait.
            # The extra matmuls burn the otherwise-dead startup window (PE
            # waits ~10us for the first gather) to trip the PE_HAM activity
            # monitor to full clock before real work arrives.
            for w in range(16):
                nc.tensor.matmul(
                    out=acc[0][:, 0:256],
                    lhsT=trilT_sb,
                    rhs=masks_sb[:, 0:256],
                    start=True,
                    stop=True,
                    skip_group_check=True,
                )
            scratch = cpool.tile([P, 1], mybir.dt.float32)
            nc.scalar.activation(
                out=scratch[:],
                in_=invd_sb[:, 0:1],
                func=mybir.ActivationFunctionType.Copy,
            )
            scratch2 = cpool.tile([P, 1], mybir.dt.float32)
            nc.vector.tensor_scalar_mul(scratch2[:], invd_sb[:, 0:1], invd_sb[:, 0:1])

            half = v // 2
            hchunk = nchunk // 2
            for k in range(nblk):
                x = xpool.tile([P, v], mm_dt)
                nc.gpsimd.indirect_dma_start(
                    out=x[:],
                    out_offset=None,
                    in_=emb[:],
                    in_offset=bass.IndirectOffsetOnAxis(
                        ap=idx_sb[:, k : k + 1], axis=0
                    ),
                )
                o = opool.tile([P, v], mm_dt)
                for c in range(nchunk):
                    nc.tensor.matmul(
                        out=acc[c][:],
                        lhsT=trilT_sb,
                        rhs=x[:, bass.ts(c, chunk)],
                        start=(k == 0),
                        stop=True,
                        skip_group_check=True,
                    )
                for c in range(nchunk):
                    sl = bass.ts(c, chunk)
                    # Chunks 0..3 -> scalar engine, 4..7 -> vector engine:
                    # each output half-tile keeps a single writer, and the
                    # per-block copy load splits across both engines.
                    if c < hchunk:
                        nc.scalar.activation(
                            out=o[:, sl],
                            in_=acc[c][:],
                            func=mybir.ActivationFunctionType.Copy,
                            scale=invd_sb[:, k : k + 1],
                        )
                    else:
                        nc.vector.tensor_scalar_mul(
                            o[:, sl], acc[c][:], invd_sb[:, k : k + 1]
                        )
                if k < nblk - 1:
                    for c in range(nchunk):
                        nc.tensor.matmul(
                            out=acc[c][:],
                            lhsT=strictT_sb,
                            rhs=x[:, bass.ts(c, chunk)],
                            start=False,
                            stop=True,
                            skip_group_check=True,
                        )
                for h in range(2):
                    csl = slice(h * half, (h + 1) * half)
                    nc.sync.dma_start(
                        out=out[bass.ts(k, P), csl], in_=o[:, csl]
                    )
                # Dead write into the just-shipped tile: routes the output
                # DMA's completion through the half's writer engine, so the
                # pool-slot reuse a few blocks later costs the next
                # scale-copy no extra sync wait (1-wait-per-instruction
                # limit).
                nc.scalar.activation(
                    out=o[:, 0:1],
                    in_=invd_sb[:, 0:1],
                    func=mybir.ActivationFunctionType.Copy,
                )
                nc.vector.tensor_scalar_mul(
                    o[:, half : half + 1], invd_sb[:, 0:1], invd_sb[:, 0:1]
                )
    nc.finalize()
    return nc


def host_inputs(idx_row, emb_bf16, t=T, v=V):
    """Per-core input map for one batch row. idx_row: [t] int, emb_bf16: [v, v]."""
    nblk = t // P
    idx32 = np.ascontiguousarray(
        np.asarray(idx_row, dtype=np.int32).reshape(nblk, P).T
    )
    invd = np.ascontiguousarray(
        (1.0 / np.arange(1, t + 1, dtype=np.float64))
        .astype(np.float32)
        .reshape(nblk, P)
        .T
    )
    masks = np.concatenate(
        [
            np.triu(np.ones((P, P), dtype=ml_dtypes.bfloat16)),
            np.tril(np.ones((P, P), dtype=ml_dtypes.bfloat16), -1),
        ],
        axis=1,
    )
    return {
        "emb": emb_bf16,
        "idx": idx32,
        "invd": invd,
        "masks": np.ascontiguousarray(masks),
    }


_nc_cache = {}


def kernel(idx, emb, _trace=False):
    from concourse.bass_utils import run_bass_kernel_spmd

    key = "nc"
    if key not in _nc_cache:
        _nc_cache[key] = build_bass()
    nc = _nc_cache[key]

    idx = np.asarray(idx)
    emb_bf16 = np.ascontiguousarray(np.asarray(emb).astype(ml_dtypes.bfloat16))
    in_maps = [host_inputs(idx[b], emb_bf16) for b in range(N_CORES)]
    res = run_bass_kernel_spmd(nc, in_maps, list(range(N_CORES)), trace=_trace)
    kernel.last_results = res
    out = np.concatenate(
        [np.asarray(r["out"]).astype(np.float32) for r in res.results], axis=0
    )
    return out
